# revision 26
# baseline (speedup 1.0000x reference)
"""Bahdanau attention kernel for Trainium2 (8 NeuronCores, data-parallel over batch).

Computes, for each batch row b:
    energy  = tanh(enc[b] @ W_e.T + (h[b] @ W_h.T) + b_attn)   # [S, DEC]
    scores  = energy @ v                                        # [S]
    out[b]  = softmax(scores)

Shapes (hardcoded): B=32, S=4096, ENC=512, DEC=512. 8 cores, 4 batch rows/core.

Device-side design (per core):
  - encoder outputs host-pre-tiled as [b, pr, p, k, s] in fp8 e4m3; W_e scaled
    x32 into fp8 so both operands qualify for the PE's DoubleRow perf mode
    (2 fp8 MACs/cell/cycle, K=256 per instruction, ~109ns per [128,256] out
    tile = 2x the bf16 rate). tanh's scale=1/32 undoes the weight scaling.
  - main matmul: pp[d_chunk(128), 256] += sum over the 2 k-planes of
    W_e8[kp].T @ enc8[kp]; 2 DoubleRow instructions cover K=512. One psum
    accumulation group per 2KB bank (start on first write, stop on last).
  - decoder projection W_h@h + b_attn (2M MACs) is computed on the host and
    shipped as a [128, DC*BPC] f32 bias table; ACT's per-partition bias
    port applies it inside the tanh.
  - ACT fuses scale + bias + tanh over a 2-bank [128,1024] PSUM pair; the
    act table is pre-warmed off a memset so the 1.3us load hides in the
    DMA head; junk matmuls ramp the PE pstate during the same window.
  - v-dot (bf16): all 4 batch rows accumulate into ONE psum tile at
    partition rows 32*b via tile_position column groups, which lets the
    4 batches' matmuls overlap inside the PE array and makes exp a single
    [128,512] instruction; row-sums run on the idle DVE.
  - softmax tail: one reduce over the 8 partial sums, one reciprocal, a
    DVE/ACT-split normalize, and 2 strided output DMA descriptors.
  - this walrus build allows one sync wait per instruction; the dataflow is
    engineered for that and a post-pass splits leftovers into wait-only drains.
  - head DMAs are descriptor-row-bound (~50ns/partition-row, ~3 parallel
    queues): pk8/bias/enc0 ride distinct trigger queues (sync/scalar/gpsimd);
    enc tiles 1-5 chain behind tile 0 via claim writes.
"""

import os
import sys

import numpy as np

try:
    import concourse.bass as bass  # noqa: F401
except ImportError:  # toolchain lives in the trn_rl repo
    for p in ("/opt/trn_rl_repo", "/root/.axon_site/_ro/trn_rl_repo"):
        if os.path.isdir(p) and p not in sys.path:
            sys.path.insert(0, p)
    import concourse.bass as bass  # noqa: F401

import ml_dtypes

B, S, ENC, DEC = 32, 4096, 512, 512
N_CORES = 8
BPC = B // N_CORES          # batch rows per core
SG = 512                    # s-columns per v-dot / psum bank
SG2 = 2 * SG                # s-columns per DMA tile
N_PR = S // SG2             # 4 s-group pairs
KC = ENC // 128             # 4 contraction chunks
KP = KC // 2                # 2 DoubleRow k-pairs
NQ = SG2 // 256             # 4 moving quarters per tile
DC = DEC // 128             # 4 output-dim chunks

W_SCALE = 32.0              # fp8 weight pre-scale, undone by tanh's scale=

# bf16 packed constant layout: [128, KC, NPKB] -- just the v chunks now
_V0 = 0             # v                col 0
NPKB = 1

_BF16 = ml_dtypes.bfloat16
_F8 = ml_dtypes.float8_e4m3

_nc_cache = None
last_results = None         # BassKernelResults of the most recent run (for test.py)


def _build_bass():
    import concourse.tile as tile
    from concourse import mybir

    f32 = mybir.dt.float32
    bf16 = mybir.dt.bfloat16
    f8 = mybir.dt.float8e4
    Act = mybir.ActivationFunctionType
    DR = mybir.MatmulPerfMode.DoubleRow

    nc = bass.Bass()

    enc_d = nc.declare_dram_parameter(
        "enc_t", [BPC, N_PR, 128, KC, SG2], f8, isOutput=False
    )
    pk8_d = nc.declare_dram_parameter("pk8", [128, KC, DEC], f8, isOutput=False)
    pkb_d = nc.declare_dram_parameter("pkb", [128, KC, NPKB], bf16, isOutput=False)
    bias_d = nc.declare_dram_parameter("bias", [128, DC * BPC], f32, isOutput=False)
    out_d = nc.declare_dram_parameter("out", [BPC, S], f32, isOutput=True)

    with tile.TileContext(nc) as tc:
        with (
            tc.tile_pool(name="consts", bufs=1) as consts,
            tc.tile_pool(name="encp", bufs=6) as encp,
            tc.tile_pool(name="enp", bufs=6) as enp,
            tc.tile_pool(name="psp", bufs=2, space="PSUM") as psp,
            tc.tile_pool(name="smp", bufs=1) as smp,
        ):
            pk8 = consts.tile([128, KC, DEC], f8)
            pkb = consts.tile([128, KC, NPKB], bf16)
            bias_sb = consts.tile([128, DC * BPC], f32)
            # DMA latency is descriptor-row-bound with ~3 parallel queues
            # (sync/scalar/gpsimd); give each head transfer its own queue
            nc.sync.dma_start(out=pk8[:], in_=pk8_d[:, :, :])
            nc.scalar.dma_start(out=bias_sb[:], in_=bias_d[:, :])

            # ACT table load + warm-up gated only on a t0 memset, so the
            # 1.3us table load runs long before the bias/pp chain is ready
            warm_f32 = smp.tile([128, 1], f32)
            nc.vector.memset(warm_f32[:, :], 0.25)
            # ~3us of junk matmuls ramp the PE pstate while the head DMAs
            # are in flight, so the first real matmuls run at full clock
            warm_bf = smp.tile([128, 256], bf16)
            nc.vector.memset(warm_bf[:, :], 0.0)
            warm_ps = psp.tile([128, SG], f32, tag="sc", name="warm_ps", bufs=2)
            for _w in range(8):
                nc.tensor.matmul(
                    warm_ps[0:1, 0:256],
                    warm_bf[:, 0:1],
                    warm_bf[:, :],
                    start=True,
                    stop=True,
                )
            act_warm = consts.tile([128, 2], f32)
            nc.scalar.activation(
                act_warm[:, 0:1], warm_f32[:, 0:1], func=Act.Exp,
                bias=warm_f32[:, 0:1],
            )
            nc.scalar.activation(
                act_warm[:, 1:2], warm_f32[:, 0:1], func=Act.Tanh,
                bias=warm_f32[:, 0:1],
            )

            expd = smp.tile([128, S], f32)
            sums8 = smp.tile([128, 2 * N_PR], f32)
            sums = smp.tile([128, 1], f32)
            recip = smp.tile([128, 1], f32)
            out_sb = smp.tile([128, S], f32)

            # Serialize enc DMAs: tile i+1's DMA is gated on tile i's first
            # half via a claim write, so first-needed data lands first
            # instead of round-robining 4+MB of traffic.
            # pk8 + enc tile 0 + pkb all launch at t0 on parallel queues
            # (the first matmul needs pk8+enc0h1, psd needs pkb); tiles 1-5
            # chain each behind the previous tile's FIRST half so at most
            # ~2 enc streams are in flight while the prelude loads race.
            first_tiles = []
            prev_gate = None
            for i in range(6):
                t = encp.tile([128, KC, SG2], f8, tag="enc_tile", name=f"enc_first{i}")
                if prev_gate is not None:
                    nc.vector.tensor_copy(t[:, 0, 0:1], prev_gate)
                    nc.vector.tensor_copy(t[:, 2, 0:1], prev_gate)
                b0, pr0 = i % BPC, i // BPC
                # tile 0: balance the 4 head transfers' descriptor rows
                # across the 3 DMA queues (sync carries pk8 + half of
                # enc0h2, gpsimd enc0h1 + the other half, scalar bias)
                if i == 0:
                    nc.gpsimd.dma_start(out=t[:, 0:2, :], in_=enc_d[b0, pr0, :, 0:2, :])
                    nc.gpsimd.dma_start(
                        out=t[0:64, 2:4, :], in_=enc_d[b0, pr0, 0:64, 2:4, :]
                    )
                    nc.sync.dma_start(
                        out=t[64:128, 2:4, :], in_=enc_d[b0, pr0, 64:128, 2:4, :]
                    )
                else:
                    nc.sync.dma_start(out=t[:, 0:2, :], in_=enc_d[b0, pr0, :, 0:2, :])
                    nc.sync.dma_start(out=t[:, 2:4, :], in_=enc_d[b0, pr0, :, 2:4, :])
                first_tiles.append(t)
                prev_gate = t[:, 0, 0:1]
            # pkb (just the v column) is not needed until the first v-dots
            nc.sync.dma_start(out=pkb[:], in_=pkb_d[:, :, :])

            # the decoder projection W_h@h + b_attn is 2M MACs -- computed
            # on the host and shipped as a tiny f32 bias table (DMA'd above)
            # re-materialize the bias through ACT: tanh's bias dep becomes
            # a same-queue edge (single-sync-wait constraint)
            bias_act = consts.tile([128, DC * BPC], f32)
            nc.scalar.copy(bias_act[:, :], bias_sb[:, :])
            it = 0
            for pr in range(N_PR):
                en_tiles = []
                for b in range(BPC):
                    if it < 6:
                        enc_tile = first_tiles[it]
                    else:
                        enc_tile = encp.tile(
                            [128, KC, SG2], f8, tag="enc_tile", name="enc_tile"
                        )
                    preloaded = it < 6
                    it += 1
                    if b == 0:
                        # per-pr score tiles: all 4 batches accumulate into
                        # partition rows 32*b; memset initializes the unused
                        # rows so the batched exp reads defined zeros
                        scts = []
                        for _h in range(2):
                            t_sc = psp.tile([128, SG], f32, tag="sc", name="sct", bufs=2)
                            nc.vector.memset(t_sc[:, :], 0.0)
                            scts.append(t_sc)
                    if not preloaded:
                        nc.sync.dma_start(
                            out=enc_tile[:, 0:2, :], in_=enc_d[b, pr, :, 0:2, :]
                        )
                        nc.sync.dma_start(
                            out=enc_tile[:, 2:4, :], in_=enc_d[b, pr, :, 2:4, :]
                        )
                    # spare last column keeps the claim write disjoint from tanh
                    en_tile = enp.tile(
                        [128, DC, SG2 + 1], bf16, tag="en_tile", name="en_tile"
                    )
                    # claim the recycled slot: carries the slot-release wait alone
                    nc.vector.tensor_copy(en_tile[:, 0, SG2 : SG2 + 1], bias_sb[:, 0:1])
                    en_tiles.append(en_tile)
                    for c in range(DC):
                        pp = psp.tile([128, 2, SG], f32, tag="proj", name="pp", bufs=3)
                        # one accumulation group per psum BANK (2KB zero
                        # region): start on the bank's first write (kp0,
                        # even q), stop on its last (kp1, odd q). kp outer
                        # keeps weights identical across 4 consecutive
                        # matmuls so ldw dedup drops 6 of 8 reloads.
                        for kp in range(KP):
                            for q in range(NQ):
                                nc.tensor.matmul(
                                    pp[:, q // 2, (q % 2) * 256 : (q % 2) * 256 + 256],
                                    pk8[:, 2 * kp : 2 * kp + 2, c * 128 : (c + 1) * 128],
                                    enc_tile[:, 2 * kp : 2 * kp + 2, q * 256 : (q + 1) * 256],
                                    start=(kp == 0 and q % 2 == 0),
                                    stop=(kp == KP - 1 and q % 2 == 1),
                                    perf_mode=DR,
                                )
                        nc.scalar.activation(
                            out=en_tile[:, c, 0:SG2],
                            in_=pp[:, :, :],
                            func=Act.Tanh,
                            bias=bias_act[:, c * BPC + b : c * BPC + b + 1],
                            scale=1.0 / W_SCALE,
                        )
                    if pr == N_PR - 1 and b == 2:
                        # last pair: batches 0-2 share one interleaved
                        # (array-parallel) v-dot block once b2's energy is
                        # done; b3's stays inline below so only its scores
                        # trail into the softmax tail
                        for half in range(2):
                            for c in range(DC):
                                for b2 in range(3):
                                    nc.tensor.matmul(
                                        scts[half][32 * b2 : 32 * b2 + 1, :],
                                        pkb[:, c, _V0 : _V0 + 1],
                                        en_tiles[b2][:, c, half * SG : (half + 1) * SG],
                                        start=(c == 0),
                                        stop=(c == DC - 1),
                                        tile_position=(0, 32 * b2),
                                        skip_group_check=True,
                                    )
                    if pr == N_PR - 1 and b == 3:
                        # half-outer so exp(h0) overlaps the h1 v-dots
                        for half in range(2):
                            for c in range(DC):
                                nc.tensor.matmul(
                                    scts[half][32 * b : 32 * b + 1, :],
                                    pkb[:, c, _V0 : _V0 + 1],
                                    en_tile[:, c, half * SG : (half + 1) * SG],
                                    start=(c == 0),
                                    stop=(c == DC - 1),
                                    tile_position=(0, 32 * b),
                                    skip_group_check=True,
                                )
                if pr != N_PR - 1:
                    # batch-interleaved v-dots: consecutive matmuls hit
                    # distinct PE column groups (tile_position 0/32/64/96)
                    # and so overlap inside the array. Groups interleave in
                    # one bank on distinct partition rows -- safe on hw
                    # (zero regions are per partition row); only the sim's
                    # collapsed group flags object, hence skip_group_check.
                    for half in range(2):
                        for c in range(DC):
                            for b2 in range(BPC):
                                nc.tensor.matmul(
                                    scts[half][32 * b2 : 32 * b2 + 1, :],
                                    pkb[:, c, _V0 : _V0 + 1],
                                    en_tiles[b2][:, c, half * SG : (half + 1) * SG],
                                    start=(c == 0),
                                    stop=(c == DC - 1),
                                    tile_position=(0, 32 * b2),
                                    skip_group_check=True,
                                )
                for half in range(2):
                    sg = 2 * pr + half
                    nc.scalar.activation(
                        out=expd[:, sg * SG : (sg + 1) * SG],
                        in_=scts[half][:, :],
                        func=Act.Exp,
                    )
                    # row-sum on the idle DVE keeps the 183ns accumulator
                    # read off the critical ACT queue
                    nc.vector.reduce_sum(
                        sums8[:, sg : sg + 1],
                        expd[:, sg * SG : (sg + 1) * SG],
                        axis=mybir.AxisListType.X,
                    )

            # softmax tail: one chain over all four batches; normalize is
            # split DVE (low half) / ACT copy-with-scale (high half) so the
            # two engines overlap, with output DMAs fired per half
            nc.vector.reduce_sum(sums[:, :], sums8[:, :], axis=mybir.AxisListType.X)
            nc.vector.reciprocal(recip[:, :], sums[:, :])
            H = 2816  # DVE ~0.63ns/col vs ACT ~0.9ns+bubble: split 2816/1280
            nc.scalar.activation(
                out=out_sb[:, H:S],
                in_=expd[:, H:S],
                func=Act.Copy,
                scale=recip[:, 0:1],
            )
            nc.vector.tensor_scalar_mul(
                out=out_sb[:, 0:H], in0=expd[:, 0:H], scalar1=recip[:, :]
            )
            # one strided descriptor per half (4 partition rows at stride
            # 32), triggers on separate queues so they fire concurrently
            nc.gpsimd.dma_start(out=out_d[0:BPC, H:S], in_=out_sb[0:97:32, H:S])
            nc.sync.dma_start(out=out_d[0:BPC, 0:H], in_=out_sb[0:97:32, 0:H])

    _split_multi_waits(nc)
    return nc


def _split_multi_waits(nc):
    """This walrus build allows ONE sync wait per instruction. The kernel body
    is engineered to respect that; Tile's auto-emitted tail drain is not (it
    waits on every processor). Split any multi-wait instruction into a chain
    of single-wait drains on the same engine followed by the original."""
    from concourse import mybir

    for bb in nc.main_func.blocks:
        new_insts = []
        for ins in bb.instructions:
            si = getattr(ins, "sync_info", None)
            if si is not None and si.on_wait and len(si.on_wait) > 1:
                waits = list(si.on_wait)
                for w in waits[:-1]:
                    d = mybir.InstNoOp(
                        name=nc.get_next_instruction_name(),
                        ins=[],
                        outs=[],
                    )
                    d.engine = ins.engine
                    d.sync_info = mybir.SyncInfo(on_wait=[w], on_update=[])
                    nc.register_instruction(d)
                    new_insts.append(d)
                si.on_wait = waits[-1:]
            new_insts.append(ins)
        bb.instructions[:] = new_insts


def _get_nc():
    global _nc_cache
    if _nc_cache is None:
        _nc_cache = _build_bass()
    return _nc_cache


def _prep_in_maps(decoder_hidden, encoder_outputs, W_attn, b_attn, v):
    decoder_hidden = np.asarray(decoder_hidden, dtype=np.float32)
    encoder_outputs = np.asarray(encoder_outputs, dtype=np.float32)
    W_attn = np.asarray(W_attn, dtype=np.float32)
    b_attn = np.asarray(b_attn, dtype=np.float32)
    v = np.asarray(v, dtype=np.float32)

    W_h = W_attn[:, :DEC]           # [d_out, d_in]
    W_e = W_attn[:, DEC:]           # [d_out, e]

    pk8 = np.ascontiguousarray(
        (W_e.T * W_SCALE).astype(_F8).reshape(KC, 128, DEC).transpose(1, 0, 2)
    )

    pkb = np.zeros((128, KC, NPKB), dtype=_BF16)
    pkb[:, :, _V0] = v.astype(_BF16).reshape(DC, 128).T

    # decoder projection + b_attn on host (2M MACs): bias[p, c*BPC+b]
    dec_proj = decoder_hidden @ W_h.T + b_attn       # [B, DEC]

    # [B, S, E] -> [B, N_PR, 128(p=e%128), KC(e//128), SG2(s)] in fp8
    enc_bt = np.ascontiguousarray(
        encoder_outputs.reshape(B, N_PR, SG2, KC, 128)
        .transpose(0, 1, 4, 3, 2)
        .astype(_F8)
    )

    in_maps = []
    for core in range(N_CORES):
        sl = slice(core * BPC, (core + 1) * BPC)
        bias = np.ascontiguousarray(
            dec_proj[sl].reshape(BPC, DC, 128).transpose(2, 1, 0)
            .reshape(128, DC * BPC)
        ).astype(np.float32)
        in_maps.append({"enc_t": enc_bt[sl], "pk8": pk8, "pkb": pkb, "bias": bias})
    return in_maps


def _ensure_ntff_hook():
    """The agent image's ``antenv`` lacks ``axon_hooks``; synthesize it with a
    ctypes-based NTFF profile hook against the injected libaxon (trace runs only)."""
    try:
        from antenv.axon_hooks import get_axon_ntff_profile_hook  # noqa: F401

        return
    except ImportError:
        pass

    import contextlib
    import ctypes
    import types

    so_path = "/opt/axon/libaxon_pjrt.so"
    hook = None
    if os.path.exists(so_path):
        lib = ctypes.CDLL(so_path)
        if hasattr(lib, "axon_start_nrt_profile"):
            lib.axon_start_nrt_profile.argtypes = [
                ctypes.POINTER(ctypes.c_int64),
                ctypes.c_size_t,
            ]
            lib.axon_start_nrt_profile.restype = ctypes.c_int64
            lib.axon_stop_nrt_profile.argtypes = [ctypes.c_char_p]
            lib.axon_stop_nrt_profile.restype = ctypes.c_int64

            @contextlib.contextmanager
            def _hook(output_dir, device_ids):
                import jax

                jax.devices()
                if device_ids:
                    ids = (ctypes.c_int64 * len(device_ids))(*device_ids)
                    rc = lib.axon_start_nrt_profile(ids, len(device_ids))
                else:
                    rc = lib.axon_start_nrt_profile(None, 0)
                if rc != 0:
                    raise RuntimeError(f"axon_start_nrt_profile rc={rc}")
                try:
                    yield
                finally:
                    n = lib.axon_stop_nrt_profile(str(output_dir).encode())
                    if n <= 0:
                        print(f"ntff capture wrote {n} files", file=sys.stderr)

            hook = _hook

    holder = {"h": hook}
    mod = types.ModuleType("antenv.axon_hooks")
    mod.get_axon_ntff_profile_hook = lambda: holder["h"]
    mod.set_axon_ntff_profile_hook = lambda h: holder.__setitem__("h", h)
    sys.modules["antenv.axon_hooks"] = mod
    import antenv

    antenv.axon_hooks = mod


def _enable_ldw_opt():
    """Turn on walrus's LDWEIGHTS dedup (consecutive same-weight matmuls skip
    the reload). Off by default in this toolchain."""
    import concourse.bass_utils as bu

    if getattr(bu, "_ldw_opt_patched", False):
        return
    orig = bu.bir_verify_and_optimise

    def patched(*args, **kw):
        import concourse.bass_utils as _b

        run0 = _b.run_command

        def run_patched(argv, **rkw):
            argv = [a.replace("--enable-ldw-opt=false", "--enable-ldw-opt=true")
                    if isinstance(a, str) else a for a in argv]
            return run0(argv, **rkw)

        _b.run_command = run_patched
        try:
            return orig(*args, **kw)
        finally:
            _b.run_command = run0

    bu.bir_verify_and_optimise = patched
    # bass2jax binds compile_bir_kernel which calls _compile_bir_impl ->
    # bir_verify_and_optimise as a module global, so this is enough.
    bu._ldw_opt_patched = True


def kernel(decoder_hidden, encoder_outputs, W_attn, b_attn, v):
    global last_results
    import concourse.bass_utils as bass_utils
    from concourse.bass_utils import run_bass_kernel_spmd

    # walrus's LDW dedup is incompatible with DoubleRow ldweights
    # (codegen refuses); keep it off unless explicitly requested
    if os.environ.get("BAHDANAU_LDW_OPT", "0") == "1":
        _enable_ldw_opt()

    nc = _get_nc()
    in_maps = _prep_in_maps(decoder_hidden, encoder_outputs, W_attn, b_attn, v)

    trace = os.environ.get("BAHDANAU_TRACE", "0") == "1"
    kwargs = {}
    if trace:
        _ensure_ntff_hook()
        bass_utils.upload_artifacts = lambda tmpdir: str(tmpdir)  # no bucket here
        kwargs["trace"] = True
        tmpdir = os.environ.get("BAHDANAU_TRACE_DIR")
        if tmpdir:
            import uuid

            tmpdir = os.path.join(tmpdir, uuid.uuid4().hex[:8])
            os.makedirs(tmpdir, exist_ok=True)
            kwargs["tmpdir"] = tmpdir

    res = run_bass_kernel_spmd(nc, in_maps, core_ids=list(range(N_CORES)), **kwargs)
    last_results = res
    out = np.concatenate([res.results[c]["out"] for c in range(N_CORES)], axis=0)
    return out.astype(np.float32)


# revision 27
# speedup vs baseline: 1.0172x; 1.0172x over previous
"""Bahdanau attention kernel for Trainium2 (8 NeuronCores, data-parallel over batch).

Computes, for each batch row b:
    energy  = tanh(enc[b] @ W_e.T + (h[b] @ W_h.T) + b_attn)   # [S, DEC]
    scores  = energy @ v                                        # [S]
    out[b]  = softmax(scores)

Shapes (hardcoded): B=32, S=4096, ENC=512, DEC=512. 8 cores, 4 batch rows/core.

Device-side design (per core):
  - encoder outputs host-pre-tiled as [b, pr, p, k, s] in fp8 e4m3; W_e scaled
    x32 into fp8 so both operands qualify for the PE's DoubleRow perf mode
    (2 fp8 MACs/cell/cycle, K=256 per instruction, ~109ns per [128,256] out
    tile = 2x the bf16 rate). tanh's scale=1/32 undoes the weight scaling.
  - main matmul: pp[d_chunk(128), 256] += sum over the 2 k-planes of
    W_e8[kp].T @ enc8[kp]; 2 DoubleRow instructions cover K=512. One psum
    accumulation group per 2KB bank (start on first write, stop on last).
  - decoder projection W_h@h + b_attn (2M MACs) is computed on the host and
    shipped as a [128, DC*BPC] f32 bias table; ACT's per-partition bias
    port applies it inside the tanh.
  - ACT fuses scale + bias + tanh over a 2-bank [128,1024] PSUM pair; the
    act table is pre-warmed off a memset so the 1.3us load hides in the
    DMA head; junk matmuls ramp the PE pstate during the same window.
  - v-dot (bf16): all 4 batch rows accumulate into ONE psum tile at
    partition rows 32*b via tile_position column groups, which lets the
    4 batches' matmuls overlap inside the PE array and makes exp a single
    [128,512] instruction; row-sums run on the idle DVE.
  - softmax tail: one reduce over the 8 partial sums, one reciprocal, a
    DVE/ACT-split normalize, and 2 strided output DMA descriptors.
  - this walrus build allows one sync wait per instruction; the dataflow is
    engineered for that and a post-pass splits leftovers into wait-only drains.
  - head DMAs are descriptor-row-bound (~50ns/partition-row, ~3 parallel
    queues): pk8/bias/enc0 ride distinct trigger queues (sync/scalar/gpsimd);
    enc tiles 1-5 chain behind tile 0 via claim writes.
"""

import os
import sys

import numpy as np

try:
    import concourse.bass as bass  # noqa: F401
except ImportError:  # toolchain lives in the trn_rl repo
    for p in ("/opt/trn_rl_repo", "/root/.axon_site/_ro/trn_rl_repo"):
        if os.path.isdir(p) and p not in sys.path:
            sys.path.insert(0, p)
    import concourse.bass as bass  # noqa: F401

import ml_dtypes

B, S, ENC, DEC = 32, 4096, 512, 512
N_CORES = 8
BPC = B // N_CORES          # batch rows per core
SG = 512                    # s-columns per v-dot / psum bank
SG2 = 2 * SG                # s-columns per DMA tile
N_PR = S // SG2             # 4 s-group pairs
KC = ENC // 128             # 4 contraction chunks
KP = KC // 2                # 2 DoubleRow k-pairs
NQ = SG2 // 256             # 4 moving quarters per tile
DC = DEC // 128             # 4 output-dim chunks

W_SCALE = 32.0              # fp8 weight pre-scale, undone by tanh's scale=

# bf16 packed constant layout: [128, KC, NPKB] -- just the v chunks now
_V0 = 0             # v                col 0
NPKB = 1

_BF16 = ml_dtypes.bfloat16
_F8 = ml_dtypes.float8_e4m3

_nc_cache = None
last_results = None         # BassKernelResults of the most recent run (for test.py)


def _build_bass():
    import concourse.tile as tile
    from concourse import mybir

    f32 = mybir.dt.float32
    bf16 = mybir.dt.bfloat16
    f8 = mybir.dt.float8e4
    Act = mybir.ActivationFunctionType
    DR = mybir.MatmulPerfMode.DoubleRow

    nc = bass.Bass()

    enc_d = nc.declare_dram_parameter(
        "enc_t", [BPC, N_PR, 128, KC, SG2], f8, isOutput=False
    )
    pk8_d = nc.declare_dram_parameter("pk8", [128, KC, DEC], f8, isOutput=False)
    pkb_d = nc.declare_dram_parameter("pkb", [128, KC, NPKB], bf16, isOutput=False)
    bias_d = nc.declare_dram_parameter("bias", [128, DC * BPC], f32, isOutput=False)
    out_d = nc.declare_dram_parameter("out", [BPC, S], f32, isOutput=True)

    with tile.TileContext(nc) as tc:
        with (
            tc.tile_pool(name="consts", bufs=1) as consts,
            tc.tile_pool(name="encp", bufs=6) as encp,
            tc.tile_pool(name="enp", bufs=6) as enp,
            tc.tile_pool(name="psp", bufs=2, space="PSUM") as psp,
            tc.tile_pool(name="smp", bufs=1) as smp,
        ):
            pk8 = consts.tile([128, KC, DEC], f8)
            pkb = consts.tile([128, KC, NPKB], bf16)
            bias_sb = consts.tile([128, DC * BPC], f32)
            # DMA latency is descriptor-row-bound with ~3 parallel queues
            # (sync/scalar/gpsimd); give each head transfer its own queue
            nc.sync.dma_start(out=pk8[:], in_=pk8_d[:, :, :])
            nc.scalar.dma_start(out=bias_sb[:], in_=bias_d[:, :])

            # ACT table load + warm-up gated only on a t0 memset, so the
            # 1.3us table load runs long before the bias/pp chain is ready
            warm_f32 = smp.tile([128, 1], f32)
            nc.vector.memset(warm_f32[:, :], 0.25)
            # ~3us of junk matmuls ramp the PE pstate while the head DMAs
            # are in flight, so the first real matmuls run at full clock
            warm_bf = smp.tile([128, 256], bf16)
            nc.vector.memset(warm_bf[:, :], 0.0)
            warm_ps = psp.tile([128, SG], f32, tag="sc", name="warm_ps", bufs=2)
            for _w in range(8):
                nc.tensor.matmul(
                    warm_ps[0:1, 0:256],
                    warm_bf[:, 0:1],
                    warm_bf[:, :],
                    start=True,
                    stop=True,
                )
            act_warm = consts.tile([128, 2], f32)
            nc.scalar.activation(
                act_warm[:, 0:1], warm_f32[:, 0:1], func=Act.Exp,
                bias=warm_f32[:, 0:1],
            )
            nc.scalar.activation(
                act_warm[:, 1:2], warm_f32[:, 0:1], func=Act.Tanh,
                bias=warm_f32[:, 0:1],
            )

            expd = smp.tile([128, S], f32)
            sums8 = smp.tile([128, 2 * N_PR], f32)
            sums = smp.tile([128, 1], f32)
            recip = smp.tile([128, 1], f32)
            out_sb = smp.tile([128, S], f32)

            # Serialize enc DMAs: tile i+1's DMA is gated on tile i's first
            # half via a claim write, so first-needed data lands first
            # instead of round-robining 4+MB of traffic.
            # pk8 + enc tile 0 + pkb all launch at t0 on parallel queues
            # (the first matmul needs pk8+enc0h1, psd needs pkb); tiles 1-5
            # chain each behind the previous tile's FIRST half so at most
            # ~2 enc streams are in flight while the prelude loads race.
            first_tiles = []
            prev_gate = None
            for i in range(6):
                t = encp.tile([128, KC, SG2], f8, tag="enc_tile", name=f"enc_first{i}")
                if prev_gate is not None:
                    nc.vector.tensor_copy(t[:, 0, 0:1], prev_gate)
                    nc.vector.tensor_copy(t[:, 2, 0:1], prev_gate)
                b0, pr0 = i % BPC, i // BPC
                # tile 0 rides the GpSimd queue, parallel to pk8/bias
                eng = nc.gpsimd if i == 0 else nc.sync
                eng.dma_start(out=t[:, 0:2, :], in_=enc_d[b0, pr0, :, 0:2, :])
                eng.dma_start(out=t[:, 2:4, :], in_=enc_d[b0, pr0, :, 2:4, :])
                first_tiles.append(t)
                prev_gate = t[:, 0, 0:1]
            # pkb (just the v column) is not needed until the first v-dots
            nc.sync.dma_start(out=pkb[:], in_=pkb_d[:, :, :])

            # the decoder projection W_h@h + b_attn is 2M MACs -- computed
            # on the host and shipped as a tiny f32 bias table (DMA'd above)
            # re-materialize the bias through ACT: tanh's bias dep becomes
            # a same-queue edge (single-sync-wait constraint)
            bias_act = consts.tile([128, DC * BPC], f32)
            nc.scalar.copy(bias_act[:, :], bias_sb[:, :])
            it = 0
            for pr in range(N_PR):
                en_tiles = []
                for b in range(BPC):
                    if it < 6:
                        enc_tile = first_tiles[it]
                    else:
                        enc_tile = encp.tile(
                            [128, KC, SG2], f8, tag="enc_tile", name="enc_tile"
                        )
                    preloaded = it < 6
                    it += 1
                    if b == 0:
                        # per-pr score tiles: all 4 batches accumulate into
                        # partition rows 32*b; memset initializes the unused
                        # rows so the batched exp reads defined zeros
                        scts = []
                        for _h in range(2):
                            t_sc = psp.tile([128, SG], f32, tag="sc", name="sct", bufs=2)
                            nc.vector.memset(t_sc[:, :], 0.0)
                            scts.append(t_sc)
                    if not preloaded:
                        nc.sync.dma_start(
                            out=enc_tile[:, 0:2, :], in_=enc_d[b, pr, :, 0:2, :]
                        )
                        nc.sync.dma_start(
                            out=enc_tile[:, 2:4, :], in_=enc_d[b, pr, :, 2:4, :]
                        )
                    # spare last column keeps the claim write disjoint from tanh
                    en_tile = enp.tile(
                        [128, DC, SG2 + 1], bf16, tag="en_tile", name="en_tile"
                    )
                    # claim the recycled slot: carries the slot-release wait alone
                    nc.vector.tensor_copy(en_tile[:, 0, SG2 : SG2 + 1], bias_sb[:, 0:1])
                    en_tiles.append(en_tile)
                    for c in range(DC):
                        pp = psp.tile([128, 2, SG], f32, tag="proj", name="pp", bufs=3)
                        # one accumulation group per psum BANK (2KB zero
                        # region): start on the bank's first write (kp0,
                        # even q), stop on its last (kp1, odd q). kp outer
                        # keeps weights identical across 4 consecutive
                        # matmuls so ldw dedup drops 6 of 8 reloads.
                        for kp in range(KP):
                            for q in range(NQ):
                                nc.tensor.matmul(
                                    pp[:, q // 2, (q % 2) * 256 : (q % 2) * 256 + 256],
                                    pk8[:, 2 * kp : 2 * kp + 2, c * 128 : (c + 1) * 128],
                                    enc_tile[:, 2 * kp : 2 * kp + 2, q * 256 : (q + 1) * 256],
                                    start=(kp == 0 and q % 2 == 0),
                                    stop=(kp == KP - 1 and q % 2 == 1),
                                    perf_mode=DR,
                                )
                        nc.scalar.activation(
                            out=en_tile[:, c, 0:SG2],
                            in_=pp[:, :, :],
                            func=Act.Tanh,
                            bias=bias_act[:, c * BPC + b : c * BPC + b + 1],
                            scale=1.0 / W_SCALE,
                        )
                    if pr == N_PR - 1 and b == 2:
                        # last pair: batches 0-2 share one interleaved
                        # (array-parallel) v-dot block once b2's energy is
                        # done; b3's stays inline below so only its scores
                        # trail into the softmax tail
                        for half in range(2):
                            for c in range(DC):
                                for b2 in range(3):
                                    nc.tensor.matmul(
                                        scts[half][32 * b2 : 32 * b2 + 1, :],
                                        pkb[:, c, _V0 : _V0 + 1],
                                        en_tiles[b2][:, c, half * SG : (half + 1) * SG],
                                        start=(c == 0),
                                        stop=(c == DC - 1),
                                        tile_position=(0, 32 * b2),
                                        skip_group_check=True,
                                    )
                    if pr == N_PR - 1 and b == 3:
                        # half-outer so exp(h0) overlaps the h1 v-dots
                        for half in range(2):
                            for c in range(DC):
                                nc.tensor.matmul(
                                    scts[half][32 * b : 32 * b + 1, :],
                                    pkb[:, c, _V0 : _V0 + 1],
                                    en_tile[:, c, half * SG : (half + 1) * SG],
                                    start=(c == 0),
                                    stop=(c == DC - 1),
                                    tile_position=(0, 32 * b),
                                    skip_group_check=True,
                                )
                if pr != N_PR - 1:
                    # batch-interleaved v-dots: consecutive matmuls hit
                    # distinct PE column groups (tile_position 0/32/64/96)
                    # and so overlap inside the array. Groups interleave in
                    # one bank on distinct partition rows -- safe on hw
                    # (zero regions are per partition row); only the sim's
                    # collapsed group flags object, hence skip_group_check.
                    for half in range(2):
                        for c in range(DC):
                            for b2 in range(BPC):
                                nc.tensor.matmul(
                                    scts[half][32 * b2 : 32 * b2 + 1, :],
                                    pkb[:, c, _V0 : _V0 + 1],
                                    en_tiles[b2][:, c, half * SG : (half + 1) * SG],
                                    start=(c == 0),
                                    stop=(c == DC - 1),
                                    tile_position=(0, 32 * b2),
                                    skip_group_check=True,
                                )
                for half in range(2):
                    sg = 2 * pr + half
                    nc.scalar.activation(
                        out=expd[:, sg * SG : (sg + 1) * SG],
                        in_=scts[half][:, :],
                        func=Act.Exp,
                    )
                    # row-sum on the idle DVE keeps the 183ns accumulator
                    # read off the critical ACT queue
                    nc.vector.reduce_sum(
                        sums8[:, sg : sg + 1],
                        expd[:, sg * SG : (sg + 1) * SG],
                        axis=mybir.AxisListType.X,
                    )

            # softmax tail: one chain over all four batches; normalize is
            # split DVE (low half) / ACT copy-with-scale (high half) so the
            # two engines overlap, with output DMAs fired per half
            nc.vector.reduce_sum(sums[:, :], sums8[:, :], axis=mybir.AxisListType.X)
            nc.vector.reciprocal(recip[:, :], sums[:, :])
            H = 2816  # DVE ~0.63ns/col vs ACT ~0.9ns+bubble: split 2816/1280
            nc.scalar.activation(
                out=out_sb[:, H:S],
                in_=expd[:, H:S],
                func=Act.Copy,
                scale=recip[:, 0:1],
            )
            nc.vector.tensor_scalar_mul(
                out=out_sb[:, 0:H], in0=expd[:, 0:H], scalar1=recip[:, :]
            )
            # one strided descriptor per half (4 partition rows at stride
            # 32), triggers on separate queues so they fire concurrently
            nc.gpsimd.dma_start(out=out_d[0:BPC, H:S], in_=out_sb[0:97:32, H:S])
            nc.sync.dma_start(out=out_d[0:BPC, 0:H], in_=out_sb[0:97:32, 0:H])

    _split_multi_waits(nc)
    return nc


def _split_multi_waits(nc):
    """This walrus build allows ONE sync wait per instruction. The kernel body
    is engineered to respect that; Tile's auto-emitted tail drain is not (it
    waits on every processor). Split any multi-wait instruction into a chain
    of single-wait drains on the same engine followed by the original."""
    from concourse import mybir

    for bb in nc.main_func.blocks:
        new_insts = []
        for ins in bb.instructions:
            si = getattr(ins, "sync_info", None)
            if si is not None and si.on_wait and len(si.on_wait) > 1:
                waits = list(si.on_wait)
                for w in waits[:-1]:
                    d = mybir.InstNoOp(
                        name=nc.get_next_instruction_name(),
                        ins=[],
                        outs=[],
                    )
                    d.engine = ins.engine
                    d.sync_info = mybir.SyncInfo(on_wait=[w], on_update=[])
                    nc.register_instruction(d)
                    new_insts.append(d)
                si.on_wait = waits[-1:]
            new_insts.append(ins)
        bb.instructions[:] = new_insts


def _get_nc():
    global _nc_cache
    if _nc_cache is None:
        _nc_cache = _build_bass()
    return _nc_cache


def _prep_in_maps(decoder_hidden, encoder_outputs, W_attn, b_attn, v):
    decoder_hidden = np.asarray(decoder_hidden, dtype=np.float32)
    encoder_outputs = np.asarray(encoder_outputs, dtype=np.float32)
    W_attn = np.asarray(W_attn, dtype=np.float32)
    b_attn = np.asarray(b_attn, dtype=np.float32)
    v = np.asarray(v, dtype=np.float32)

    W_h = W_attn[:, :DEC]           # [d_out, d_in]
    W_e = W_attn[:, DEC:]           # [d_out, e]

    pk8 = np.ascontiguousarray(
        (W_e.T * W_SCALE).astype(_F8).reshape(KC, 128, DEC).transpose(1, 0, 2)
    )

    pkb = np.zeros((128, KC, NPKB), dtype=_BF16)
    pkb[:, :, _V0] = v.astype(_BF16).reshape(DC, 128).T

    # decoder projection + b_attn on host (2M MACs): bias[p, c*BPC+b]
    dec_proj = decoder_hidden @ W_h.T + b_attn       # [B, DEC]

    # [B, S, E] -> [B, N_PR, 128(p=e%128), KC(e//128), SG2(s)] in fp8
    enc_bt = np.ascontiguousarray(
        encoder_outputs.reshape(B, N_PR, SG2, KC, 128)
        .transpose(0, 1, 4, 3, 2)
        .astype(_F8)
    )

    in_maps = []
    for core in range(N_CORES):
        sl = slice(core * BPC, (core + 1) * BPC)
        bias = np.ascontiguousarray(
            dec_proj[sl].reshape(BPC, DC, 128).transpose(2, 1, 0)
            .reshape(128, DC * BPC)
        ).astype(np.float32)
        in_maps.append({"enc_t": enc_bt[sl], "pk8": pk8, "pkb": pkb, "bias": bias})
    return in_maps


def _ensure_ntff_hook():
    """The agent image's ``antenv`` lacks ``axon_hooks``; synthesize it with a
    ctypes-based NTFF profile hook against the injected libaxon (trace runs only)."""
    try:
        from antenv.axon_hooks import get_axon_ntff_profile_hook  # noqa: F401

        return
    except ImportError:
        pass

    import contextlib
    import ctypes
    import types

    so_path = "/opt/axon/libaxon_pjrt.so"
    hook = None
    if os.path.exists(so_path):
        lib = ctypes.CDLL(so_path)
        if hasattr(lib, "axon_start_nrt_profile"):
            lib.axon_start_nrt_profile.argtypes = [
                ctypes.POINTER(ctypes.c_int64),
                ctypes.c_size_t,
            ]
            lib.axon_start_nrt_profile.restype = ctypes.c_int64
            lib.axon_stop_nrt_profile.argtypes = [ctypes.c_char_p]
            lib.axon_stop_nrt_profile.restype = ctypes.c_int64

            @contextlib.contextmanager
            def _hook(output_dir, device_ids):
                import jax

                jax.devices()
                if device_ids:
                    ids = (ctypes.c_int64 * len(device_ids))(*device_ids)
                    rc = lib.axon_start_nrt_profile(ids, len(device_ids))
                else:
                    rc = lib.axon_start_nrt_profile(None, 0)
                if rc != 0:
                    raise RuntimeError(f"axon_start_nrt_profile rc={rc}")
                try:
                    yield
                finally:
                    n = lib.axon_stop_nrt_profile(str(output_dir).encode())
                    if n <= 0:
                        print(f"ntff capture wrote {n} files", file=sys.stderr)

            hook = _hook

    holder = {"h": hook}
    mod = types.ModuleType("antenv.axon_hooks")
    mod.get_axon_ntff_profile_hook = lambda: holder["h"]
    mod.set_axon_ntff_profile_hook = lambda h: holder.__setitem__("h", h)
    sys.modules["antenv.axon_hooks"] = mod
    import antenv

    antenv.axon_hooks = mod


def _enable_ldw_opt():
    """Turn on walrus's LDWEIGHTS dedup (consecutive same-weight matmuls skip
    the reload). Off by default in this toolchain."""
    import concourse.bass_utils as bu

    if getattr(bu, "_ldw_opt_patched", False):
        return
    orig = bu.bir_verify_and_optimise

    def patched(*args, **kw):
        import concourse.bass_utils as _b

        run0 = _b.run_command

        def run_patched(argv, **rkw):
            argv = [a.replace("--enable-ldw-opt=false", "--enable-ldw-opt=true")
                    if isinstance(a, str) else a for a in argv]
            return run0(argv, **rkw)

        _b.run_command = run_patched
        try:
            return orig(*args, **kw)
        finally:
            _b.run_command = run0

    bu.bir_verify_and_optimise = patched
    # bass2jax binds compile_bir_kernel which calls _compile_bir_impl ->
    # bir_verify_and_optimise as a module global, so this is enough.
    bu._ldw_opt_patched = True


def kernel(decoder_hidden, encoder_outputs, W_attn, b_attn, v):
    global last_results
    import concourse.bass_utils as bass_utils
    from concourse.bass_utils import run_bass_kernel_spmd

    # walrus's LDW dedup is incompatible with DoubleRow ldweights
    # (codegen refuses); keep it off unless explicitly requested
    if os.environ.get("BAHDANAU_LDW_OPT", "0") == "1":
        _enable_ldw_opt()

    nc = _get_nc()
    in_maps = _prep_in_maps(decoder_hidden, encoder_outputs, W_attn, b_attn, v)

    trace = os.environ.get("BAHDANAU_TRACE", "0") == "1"
    kwargs = {}
    if trace:
        _ensure_ntff_hook()
        bass_utils.upload_artifacts = lambda tmpdir: str(tmpdir)  # no bucket here
        kwargs["trace"] = True
        tmpdir = os.environ.get("BAHDANAU_TRACE_DIR")
        if tmpdir:
            import uuid

            tmpdir = os.path.join(tmpdir, uuid.uuid4().hex[:8])
            os.makedirs(tmpdir, exist_ok=True)
            kwargs["tmpdir"] = tmpdir

    res = run_bass_kernel_spmd(nc, in_maps, core_ids=list(range(N_CORES)), **kwargs)
    last_results = res
    out = np.concatenate([res.results[c]["out"] for c in range(N_CORES)], axis=0)
    return out.astype(np.float32)
